# revision 1
# baseline (speedup 1.0000x reference)
"""BTT layer on 8 Trainium2 NeuronCores.

Math:  out = X @ G + bias,  X: (8192, 4096) fp32, G: (4096, 4096) where
       G[(j,x),(y,i)] = sum_b core1[j,x,i,0,b] * core0[j,y,i,b,0]   (d=64, rank=16)

Strategy (sharding_hint option 2 - materialized G):
  - Host materializes G once (0.27 GFLOP, 0.1% of total work) and casts
    X / G to bf16 (fp32 accumulation happens in PSUM on device).
  - Data-parallel over the 8192 token rows: each of the 8 cores computes a
    (1024, 4096) output shard = X_shard @ G + bias.
  - Per core the kernel computes outT (c-major) so that matmul needs no
    on-device transpose: for each 128-wide column tile ct,
        psum[c(128), t(512)] += G_tile[k(128), c(128)].T @ XT[k(128), t(512)]
    accumulated over 32 k-tiles, then bias (per-partition scalar) is added
    while copying PSUM -> SBUF, and the (128, 1024) result is DMA'd out.
  - All DRAM operands are pre-tiled on the host so every DMA is a straight
    partition-major contiguous copy.
"""

import os

import numpy as np
import ml_dtypes

import concourse.bass as bass
import concourse.mybir as mybir
import concourse.tile as tile
from concourse import bacc
from concourse.bass_utils import run_bass_kernel_spmd

N_CORES = 8
SIZE = 4096          # model dim (k and c)
T_TOTAL = 2 * 4096   # tokens
T = T_TOTAL // N_CORES  # 1024 tokens per core
KT = SIZE // 128     # 32 k-tiles
CT = SIZE // 128     # 32 c-tiles
TCH = T // 512       # 2 moving chunks of 512 tokens

BF16 = mybir.dt.bfloat16
FP32 = mybir.dt.float32
npbf16 = ml_dtypes.bfloat16

_CACHE = {}


def _build():
    """Build + compile the per-core Bass program (shared across all 8 cores)."""
    nc = bacc.Bacc(
        "TRN2",
        target_bir_lowering=False,
        debug=False,
        num_devices=N_CORES,
        enable_partition_id=False,
    )
    # Host-pretiled layouts (partition-major so DMAs are contiguous):
    #   xt   [128 kp, KT, T]      bf16 : xt[kp, kt, t] = X_shard[t, kt*128+kp]
    #   g    [CT, 128 kp, KT, 128] bf16: g[ct, kp, kt, cp] = G[kt*128+kp, ct*128+cp]
    #   bias [128 cp, CT]          fp32: bias[cp, ct] = bias_vec[ct*128+cp]
    #   outT [CT, 128 cp, T]       fp32: outT[ct, cp, t] = out[t, ct*128+cp]
    xt_d = nc.dram_tensor("xt", (128, KT, T), BF16, kind="ExternalInput")
    g_d = nc.dram_tensor("g", (CT, 128, KT, 128), BF16, kind="ExternalInput")
    b_d = nc.dram_tensor("bias", (128, CT), FP32, kind="ExternalInput")
    out_d = nc.dram_tensor("outT", (CT, 128, T), FP32, kind="ExternalOutput")

    NG = 2  # column tiles processed per group (interleaved in the kt loop)
    with tile.TileContext(nc) as tc:
        with (
            tc.tile_pool(name="xt", bufs=1) as xpool,
            tc.tile_pool(name="g", bufs=3) as gpool,
            tc.tile_pool(name="bias", bufs=1) as bpool,
            tc.tile_pool(name="out", bufs=4) as opool,
            tc.tile_pool(name="psum", bufs=2, space="PSUM") as ppool,
        ):

            def load_g_piece(grp, kt_lo, n, whole=False, eng=None):
                """Load k-tiles [kt_lo, kt_lo+n) of one NG-wide column group
                of G as its own tile (own DMA dependency)."""
                if whole:
                    tag, bufs = "g", None
                else:
                    tag, bufs = f"g{grp}k{kt_lo}", 1
                g_sb = gpool.tile(
                    [128, NG, n, 128], BF16, name=f"g{grp}k{kt_lo}", tag=tag, bufs=bufs
                )
                (eng or nc.sync).dma_start(
                    g_sb[:],
                    g_d[grp * NG : (grp + 1) * NG, :, kt_lo : kt_lo + n, :].rearrange(
                        "t p a c -> p t a c"
                    ),
                )
                return (kt_lo, g_sb)

            def load_g(grp):
                return [load_g_piece(grp, 0, KT, whole=True)]

            def g_slice(pieces, c, kt):
                for lo, g_sb in reversed(pieces):
                    if kt >= lo:
                        return g_sb[:, c, kt - lo, :]
                raise AssertionError

            def load_xk(kt):
                xk_t = xpool.tile([128, T], BF16, name=f"xk{kt}", tag=f"xk{kt}")
                nc.sync.dma_start(xk_t[:], xt_d[:, kt, :])
                return xk_t

            # Startup choreography: the matmul stream consumes xk[kt] at
            # ~0.86us per piece while DMA delivers ~0.73us per piece, so
            # interleave the group-0 weight pieces between X pieces such that
            # every operand lands just before the PE needs it.
            # HAM pre-warm: the PE boots clock-throttled (1.2 GHz) and needs
            # ~3.4us of sustained matmul activity to unthrottle. Run dummy
            # matmuls on a zeroed scratch tile during the initial DMA wait so
            # the first real matmuls start at the warm 2.4 GHz rate. The
            # borrowed psum slot is cleared by the real chain's start=True.
            # N=128 dummies: tiny memset (0.13us) and a short per-MM duration
            # so the warm-up tail ends right at the HAM flip instead of
            # delaying the first real matmul.
            warm = xpool.tile([128, 128], BF16, name="warm", tag="warm")
            nc.vector.memset(warm[:], 0.0)
            wps = ppool.tile([128, 512], FP32, name="wps", tag="ps00")
            NWARM = 34
            for i in range(NWARM):
                nc.tensor.matmul(
                    wps[:, 0:128],
                    warm[:],
                    warm[:],
                    start=(i == 0),
                    stop=(i == NWARM - 1),
                )

            # The first four G pieces go on the scalar-engine DGE queue so
            # their ~0.65us-per-DMA issue cost runs concurrently with the X
            # piece issues on the sync queue (bulk loads stay on sync).
            xk = [None] * KT
            xk[0] = load_xk(0)
            gA, gB = [], []
            gA.append(load_g_piece(0, 0, 4, eng=nc.scalar))
            gB.append(load_g_piece(1, 0, 4, eng=nc.scalar))
            for kt in range(1, 3):
                xk[kt] = load_xk(kt)
            gA.append(load_g_piece(0, 4, 4, eng=nc.scalar))
            gB.append(load_g_piece(1, 4, 4, eng=nc.scalar))
            for kt in range(3, 6):
                xk[kt] = load_xk(kt)
            gA.append(load_g_piece(0, 8, 8))
            gB.append(load_g_piece(1, 8, 8))
            for kt in range(6, 16):
                xk[kt] = load_xk(kt)
            # gA3/gB3 cover kt16-31 (first needed ~38.7us on the pre-warmed
            # timeline); issuing them after xk15 keeps xk11-15 ahead of the
            # ~1.73us/piece superblock consumption rate.
            gA.append(load_g_piece(0, 16, 16))
            gB.append(load_g_piece(1, 16, 16))
            for kt in range(16, KT):
                xk[kt] = load_xk(kt)
            b_sb = bpool.tile([128, CT], FP32)
            nc.sync.dma_start(b_sb[:], b_d[:])
            # Whole-G prefetch for the two blocks after the superblock: their
            # in-loop issue point would be blocked behind the superblock's
            # output DMAs on the sync queue.
            g_pre = {2: load_g(2), 3: load_g(3)}

            # One 4-wide superblock first: 8 matmuls ready per arriving X
            # piece keeps the PE saturated while X streams in (DMA delivers
            # ~0.9us/piece, 2-wide consumption is only 0.86us/piece).
            blocks = [[0, 1]] + [[g] for g in range(2, CT // NG)]

            for bi, blk in enumerate(blocks):
                if blk[0] == 0:
                    g_pieces = [gA, gB]
                else:
                    g_pieces = [
                        g_pre.pop(grp) if grp in g_pre else load_g(grp)
                        for grp in blk
                    ]
                cts = [grp * NG + c for grp in blk for c in range(NG)]
                ps = [
                    [
                        ppool.tile(
                            [128, 512], FP32, name=f"ps{ci}{h}", tag=f"ps{ci % 2}{h}"
                        )
                        for h in range(TCH)
                    ]
                    for ci in range(len(cts))
                ]
                last_blk = bi == len(blocks) - 1
                if last_blk:
                    # (c, h)-major so each psum chain completes as early as
                    # possible and the output drain overlaps the final matmuls.
                    mm_order = [
                        (kt, ci, h)
                        for ci in range(len(cts))
                        for h in range(TCH)
                        for kt in range(KT)
                    ]
                else:
                    mm_order = [
                        (kt, ci, h)
                        for kt in range(KT)
                        for ci in range(len(cts))
                        for h in range(TCH)
                    ]
                for kt, ci, h in mm_order:
                    nc.tensor.matmul(
                        ps[ci][h][:],
                        g_slice(g_pieces[ci // NG], ci % NG, kt),
                        xk[kt][:, h * 512 : (h + 1) * 512],
                        start=(kt == 0),
                        stop=(kt == KT - 1),
                    )
                for ci, ct in enumerate(cts):
                    o_sb = opool.tile([128, T], FP32, name=f"o{ct}", tag="o")
                    for h in range(TCH):
                        final_chain = (
                            last_blk and ci == len(cts) - 1 and h == TCH - 1
                        )
                        if final_chain:
                            # Split the very last drain into halves on two DGE
                            # queues so the tail DVE op and DMAs pipeline.
                            for q, eng in ((0, nc.scalar), (1, nc.sync)):
                                sl = slice(h * 512 + q * 256, h * 512 + (q + 1) * 256)
                                nc.vector.tensor_scalar_add(
                                    o_sb[:, sl],
                                    ps[ci][h][:, q * 256 : (q + 1) * 256],
                                    b_sb[:, ct : ct + 1],
                                )
                                eng.dma_start(out_d[ct, :, sl], o_sb[:, sl])
                        else:
                            nc.vector.tensor_scalar_add(
                                o_sb[:, h * 512 : (h + 1) * 512],
                                ps[ci][h][:],
                                b_sb[:, ct : ct + 1],
                            )
                            nc.sync.dma_start(
                                out_d[ct, :, h * 512 : (h + 1) * 512],
                                o_sb[:, h * 512 : (h + 1) * 512],
                            )

    nc.compile()
    return nc


def _prep_inputs(x, core0, core1, bias):
    """Host-side layout prep: materialize G, cast to bf16, pre-tile."""
    # G[(j,x),(y,i)] = sum_b core1[j,x,i,0,b] * core0[j,y,i,b,0]
    c1 = np.ascontiguousarray(core1[:, :, :, 0, :])  # (j, x, i, b)
    c0 = np.ascontiguousarray(core0[:, :, :, :, 0])  # (j, y, i, b)
    G = np.einsum("jxib,jyib->jxyi", c1, c0, optimize=True).reshape(SIZE, SIZE)
    Gb = G.astype(npbf16)
    # g[ct, kp, kt, cp]
    g_dev = np.ascontiguousarray(
        Gb.reshape(KT, 128, CT, 128).transpose(2, 1, 0, 3)
    )
    bias_dev = np.ascontiguousarray(
        bias.astype(np.float32).reshape(CT, 128).T
    )

    Xf = x.reshape(T_TOTAL, SIZE)
    in_maps = []
    for c in range(N_CORES):
        shard = Xf[c * T : (c + 1) * T].astype(npbf16)  # (T, 4096)
        # xt[kp, kt, t] = shard[t, kt*128+kp]
        xt = np.ascontiguousarray(shard.T.reshape(KT, 128, T).transpose(1, 0, 2))
        in_maps.append({"xt": xt, "g": g_dev, "bias": bias_dev})
    return in_maps


def kernel(x, core0, core1, bias):
    x = np.asarray(x, dtype=np.float32)
    core0 = np.asarray(core0, dtype=np.float32)
    core1 = np.asarray(core1, dtype=np.float32)
    bias = np.asarray(bias, dtype=np.float32)

    if "nc" not in _CACHE:
        _CACHE["nc"] = _build()
    nc = _CACHE["nc"]

    in_maps = _prep_inputs(x, core0, core1, bias)
    trace = bool(int(os.environ.get("BTT_TRACE", "0")))
    res = run_bass_kernel_spmd(
        nc, in_maps, core_ids=list(range(N_CORES)), trace=trace
    )
    _CACHE["last_exec_time_ns"] = res.exec_time_ns

    out = np.empty((T_TOTAL, SIZE), dtype=np.float32)
    for c in range(N_CORES):
        outT = res.results[c]["outT"]  # (CT, 128, T)
        out[c * T : (c + 1) * T] = outT.reshape(SIZE, T).T
    return out.reshape(x.shape)

